# revision 25
# baseline (speedup 1.0000x reference)
"""Trainium2 Bass kernel for nn_Encoder_55688545960036.

Network: pointnet-style shared MLP (3->64->128, eval-mode BN folded into the
weights on the host, relu), 16 branch matmuls 128->1024 with folded BN and a
maxpool over the 2048 points of each batch element, squash over the branch
axis, capsule transform u[b,o,i,v] = sum_e caps[b,i,e] * Wc[o,i,e,v], 3 rounds
of dynamic routing, squash -> [4,32,32] output.

Distribution over 8 NeuronCores:
  phase A: branch axis k=16 -> 2 branches per core (shared MLP replicated).
  phase B: in-capsule axis i=1024 -> 128 per core (Wc 67MB -> 8.4MB/core).
  collectives: one AllToAll that converts per-core (2 branches, all 1024 i)
  feat into per-core (all 16 branches, 128-i shard), then 3 AllReduces of the
  routing partial sums s[4,32,32] (one per routing iteration). The AllReduce
  outputs use Local (not Shared) DRAM: Shared-output collectives measured
  ~158us each on this fabric vs ~17us for Local.

All heavy matmuls run in float32r (full PE rate, ~1.5e-4 rel err on HW).
The maxpool is fused into PSUM evacuation with tensor_scalar accum max.
The capsule einsum is done as 16 block-diagonal matmuls (8 capsules of the
i-shard per group, caps entries on the block diagonal) with PE column tiling.
Routing keeps (i,b) on partitions and (o,v) in the free dims so the softmax
over o is a free-axis op; sums over i (partitions) are tiny selector matmuls.
"""

import numpy as np
from contextlib import ExitStack

import concourse.bass as bass
import concourse.tile as tile
from concourse import bacc, mybir
from concourse import bass_utils

EPS = 1e-5
N_CORES = 8
B = 4
BN_ = 4 * 2048  # 8192 points
f32 = mybir.dt.float32
f32r = mybir.dt.float32r
AL = mybir.AluOpType
AF = mybir.ActivationFunctionType
AX = mybir.AxisListType
bf16 = mybir.dt.bfloat16
_BF = mybir.dt.np(bf16)

_CACHE = {}

# maxpool block schedule: 64 blocks of (k, oc, b), each with 2048 points in
# two [128,1024] PSUM tiles. Path V = DVE tensor_tensor max of both psum
# tiles (one instruction reads both) -> bf16 SBUF + remax; path A = 2 ACT
# copies -> bf16 SBUF + remax. Remax engine: 'v' = DVE (bf16 4x), 'p' =
# GpSimd/Pool. Tuned so DVE/ACT/Pool busy are balanced.
MAXPOOL_PATHS = ["D", "A", "A", "A", "D", "A", "A", "D",
                 "A", "A", "A", "D", "A", "A", "D", "A"]


def _build_bass(reps=1, debug=False, stage=4, nocoll=False):
    # stage: 1=MLP only, 2=+branch/maxpool, 3=+A2A/caps/u, 4=full (routing)
    # nocoll: replace collectives with local DRAM copies (for TimelineSim)
    # reps > 1 replicates the compute body end-to-end inside one NEFF; used
    # only for wall-clock-difference timing in the dev harness.
    nc = bacc.Bacc("TRN2", target_bir_lowering=False, debug=False,
                   num_devices=N_CORES)

    # ---- DRAM I/O ----
    d_xT = nc.dram_tensor("xT", [3, BN_], f32r, kind="ExternalInput").ap()
    d_w1f = nc.dram_tensor("w1f", [3, 64], f32r, kind="ExternalInput").ap()
    d_c1f = nc.dram_tensor("c1f", [64, 1], f32, kind="ExternalInput").ap()
    d_w2f = nc.dram_tensor("w2f", [64, 128], f32r, kind="ExternalInput").ap()
    d_c2f = nc.dram_tensor("c2f", [128, 1], f32, kind="ExternalInput").ap()
    d_wbT = nc.dram_tensor("wbT", [128, 2048], f32r, kind="ExternalInput").ap()
    d_cb = nc.dram_tensor("cb", [128, 16], f32, kind="ExternalInput").ap()
    d_wc = nc.dram_tensor("wc", [16, 128, 1024], bf16, kind="ExternalInput").ap()
    d_sel132 = nc.dram_tensor("sel132", [128, 4], bf16, kind="ExternalInput").ap()
    d_sel1 = nc.dram_tensor("sel1", [128, 4], bf16, kind="ExternalInput").ap()
    d_sel4to128 = nc.dram_tensor("sel4to128", [4, 128], f32r,
                                 kind="ExternalInput").ap()
    d_selsq = nc.dram_tensor("selsq", [64, 4], f32r, kind="ExternalInput").ap()
    d_sel4to64 = nc.dram_tensor("sel4to64", [4, 64], f32r,
                                kind="ExternalInput").ap()
    d_zeros = nc.dram_tensor("zeros512", [128, 512], bf16,
                             kind="ExternalInput").ap()
    d_ident = nc.dram_tensor("ident128", [128, 128], f32,
                             kind="ExternalInput").ap()
    d_ones = nc.dram_tensor("ones128", [128, 1], f32r,
                            kind="ExternalInput").ap()
    d_out = nc.dram_tensor("out", [B, 32, 32], f32, kind="ExternalOutput").ap()
    d_dbg = {}
    if debug:
        for nm, shp in [("h2T", [128, BN_]), ("feat", [128, 64]),
                        ("FT", [64, 128]), ("capsT", [64, 128]),
                        ("lhsT", [128, 512]), ("u0", [128, 1024]),
                        ("u1", [128, 1024]), ("u2", [128, 1024]),
                        ("u3", [128, 1024]), ("blog0", [128, 128]),
                        ("sg0", [4, 1024]), ("a0", [4, 1024]),
                        ("c1it", [128, 128]), ("sg1", [4, 1024])]:
            d_dbg[nm] = nc.dram_tensor("dbg_" + nm, shp, f32,
                                       kind="ExternalOutput").ap()

    # collective bounce buffers (internal DRAM); A2A operates on first-dim
    # blocks: in[j] goes to rank j, out[r] came from rank r.
    d_a2a_in = [nc.dram_tensor(f"a2a_in_r{r}", [8, 2, B, 128], f32)
                for r in range(reps)]
    d_a2a_out = [nc.dram_tensor(f"a2a_out_r{r}", [8, 2, B, 128], f32)
                 for r in range(reps)]
    d_s_in = [[nc.dram_tensor(f"s_in{t}_r{r}", [B, 1024], f32)
               for t in range(3)] for r in range(reps)]
    d_s_out = [[nc.dram_tensor(f"s_out{t}_r{r}", [B, 1024], f32)
               for t in range(3)] for r in range(reps)]

    rg = [list(range(N_CORES))]

    with tile.TileContext(nc) as tc, ExitStack() as ctx:
        const = ctx.enter_context(tc.tile_pool(name="const", bufs=1))
        big = ctx.enter_context(tc.tile_pool(name="big", bufs=1))
        work = ctx.enter_context(tc.tile_pool(name="work", bufs=2))
        small = ctx.enter_context(tc.tile_pool(name="small", bufs=1))

        # ---- load constants / weights ----
        def load_const(name, dram, shape, dt, eng=None):
            t = const.tile(shape, dt, name=name)
            (eng or nc.sync).dma_start(out=t, in_=dram)
            return t

        xT = load_const("xT_sb", d_xT, [3, BN_], f32r)
        w1f = load_const("w1f_sb", d_w1f, [3, 64], f32r)
        c1f = load_const("c1f_sb", d_c1f, [64, 1], f32)
        w2f = load_const("w2f_sb", d_w2f, [64, 128], f32r)
        c2f = load_const("c2f_sb", d_c2f, [128, 1], f32)
        wbT = load_const("wbT_sb", d_wbT, [128, 2048], f32r)
        cb = load_const("cb_sb", d_cb, [128, 16], f32)
        sel132 = load_const("sel132_sb", d_sel132, [128, 4], bf16)
        sel1 = load_const("sel1_sb", d_sel1, [128, 4], bf16)
        sel4to128 = load_const("sel4to128_sb", d_sel4to128, [4, 128], f32r)
        selsq = load_const("selsq_sb", d_selsq, [64, 4], f32r)
        sel4to64 = load_const("sel4to64_sb", d_sel4to64, [4, 64], f32r)

        ident = load_const("ident_sb", d_ident, [128, 128], f32)
        ones128 = load_const("ones128_sb", d_ones, [128, 1], f32r)
        wc_sb = [load_const(f"wc_sb{g}", d_wc[g], [128, 1024], bf16,
                            eng=nc.gpsimd)
                 for g in range(16)]

        def _body(rep):
            h2T = big.tile([128, BN_], f32r)  # [channel, point]

            # ---- phase A: shared MLP ----
            # 1024-wide psum tiles (2 matmuls each, matmul outputs stay
            # within a 512-col bank) halve the evacuation-op count.
            with tc.tile_pool(name="ps_mlp", bufs=2, space="PSUM") as ps_mlp:
                for j in range(8):
                    sl = bass.ts(j, 1024)
                    p1 = ps_mlp.tile([64, 1024], f32, tag="p1")
                    for q2 in range(2):
                        nc.tensor.matmul(p1[:, bass.ts(q2, 512)], w1f,
                                         xT[:, bass.ts(2 * j + q2, 512)],
                                         start=True, stop=True)
                    h1c = work.tile([64, 1024], f32r, tag="h1c", bufs=2)
                    nc.scalar.activation(out=h1c, in_=p1, func=AF.Relu,
                                         bias=c1f, scale=1.0)
                    p2 = ps_mlp.tile([128, 1024], f32, tag="p2")
                    for q2 in range(2):
                        nc.tensor.matmul(p2[:, bass.ts(q2, 512)], w2f,
                                         h1c[:, bass.ts(q2, 512)],
                                         start=True, stop=True)
                    if j % 2 == 0:
                        nc.scalar.activation(out=h2T[:, sl], in_=p2,
                                             func=AF.Relu, bias=c2f, scale=1.0)
                    else:
                        nc.vector.tensor_scalar(out=h2T[:, sl], in0=p2,
                                                scalar1=c2f, scalar2=0.0,
                                                op0=AL.add, op1=AL.max)

            if stage <= 1:
                nc.sync.dma_start(
                    d_out, h2T[0:B, 0:1024].bitcast(f32)
                    .rearrange("p (o v) -> p o v", v=32))
                return
            # ---- phase A: branch matmuls + fused maxpool ----
            feat_sb = big.tile([128, 64], f32)  # [o_in_chunk, (k, oc, b)]
            feat_pt = big.tile([128, 128], f32)  # per-half partials
            with tc.tile_pool(name="ps_y", bufs=4, space="PSUM") as ps_y:
                idx = 0
                for k in range(2):
                    for oc in range(8):
                        lw = wbT[:, bass.ts(k * 8 + oc, 128)]
                        for b in range(B):
                            for h in range(2):
                                py = ps_y.tile([128, 1024], f32, tag="py")
                                for q2 in range(2):
                                    q = 2 * h + q2
                                    nc.tensor.matmul(
                                        py[:, bass.ts(q2, 512)], lw,
                                        h2T[:, bass.ts(4 * b + q, 512)],
                                        start=True, stop=True)
                                s2 = ((oc * 2 + k) * 4 + b) * 2 + h
                                acc = feat_pt[:, s2:s2 + 1]
                                path = MAXPOOL_PATHS[idx % len(MAXPOOL_PATHS)]
                                if path == "D":
                                    nc.vector.tensor_scalar(
                                        out=py, in0=py, scalar1=-3.0e38,
                                        scalar2=None, op0=AL.max, op1=AL.max,
                                        accum_out=acc)
                                else:
                                    pair = work.tile([128, 1024], bf16,
                                                     tag="pair", bufs=8)
                                    nc.scalar.copy(pair, py)
                                    eng = (nc.vector if path == "A"
                                           else nc.gpsimd)
                                    eng.tensor_scalar(
                                        out=pair, in0=pair, scalar1=-3.0e38,
                                        scalar2=None, op0=AL.max, op1=AL.max,
                                        accum_out=acc)
                                idx += 1
            nc.vector.tensor_max(feat_sb, feat_pt[:, 0::2], feat_pt[:, 1::2])

            # feat += cb (cb[p, (oc, k)] broadcast over b)
            feat_v = feat_sb.rearrange("p (oc k b) -> p oc k b", oc=8, k=2)
            cb_bc = cb.rearrange("p (oc k) -> p oc k", oc=8).unsqueeze(3) \
                      .broadcast_to((128, 8, 2, 4))
            nc.vector.tensor_add(feat_v, feat_v, cb_bc)

            if debug and rep == 0:
                nc.sync.dma_start(d_dbg["feat"], feat_sb)
            if stage <= 2:
                nc.sync.dma_start(
                    d_out[:, 0:4, 0:8],
                    feat_sb[0:B, 0:32].rearrange("p (o v) -> p o v", v=8))
                return
            # transpose feat on the PE so the a2a_in DMA is one contiguous
            # 32KB copy (featT flat layout == a2a_in flat layout).
            with tc.tile_pool(name="ps_ft", bufs=1, space="PSUM") as ps_ft:
                p_ftr = ps_ft.tile([64, 128], f32, tag="pft")
                nc.tensor.transpose(p_ftr, feat_sb, ident)
                featT = work.tile([64, 128], f32, tag="featT")
                nc.vector.tensor_copy(featT, p_ftr)
            nc.sync.dma_start(d_a2a_in[rep].ap(), featT)

            # ---- AllToAll: out viewed [16(e), B, 128(i_local)] ----
            if nocoll:
                nc.sync.dma_start(d_a2a_out[rep].ap(), d_a2a_in[rep].ap())
            else:
                nc.gpsimd.collective_compute(
                    "AllToAll", AL.bypass, ins=[d_a2a_in[rep].ap().opt()],
                    outs=[d_a2a_out[rep].ap().opt()], replica_groups=rg)

            # ---- phase B ----
            with tc.tile_pool(name="ps_b", bufs=2, space="PSUM") as ps_b, \
                 tc.tile_pool(name="ps_tiny", bufs=2, space="PSUM") as ps_tiny, \
                 tc.tile_pool(name="ps_s", bufs=1, space="PSUM") as ps_s:

                # caps: squash over branch axis e. FT[(b,e) b-major, i_local]:
                # FT[16b+e, i] = a2a_out[e, b, i]; one DMA per b (custom
                # multi-dim APs on internal DRAM tensors get re-normalized,
                # so keep each transfer 2-dim and unambiguous).
                FT = big.tile([64, 128], f32)
                a2a_out_v = d_a2a_out[rep].ap().rearrange(
                    "r k b l -> (r k) b l")  # [16 e, 4 b, 128 i]
                for b in range(4):
                    nc.sync.dma_start(FT[16 * b:16 * (b + 1), :],
                                      a2a_out_v[:, b, :])

                # u is computed from UNNORMALIZED caps; the squash scale
                # fct[b,i] is derived from column-sums of lhsT^2 (each
                # block-diag column holds exactly one capsule vector) and
                # folded into the u evacuation, off the critical path.
                capsT = work.tile([64, 128], bf16, tag="capsT")
                nc.vector.tensor_copy(capsT, FT)
                if debug and rep == 0:
                    nc.sync.dma_start(d_dbg["FT"], FT)
                    nc.gpsimd.dma_start(out=d_dbg["capsT"], in_=capsT)

                # block-diagonal lhsT: lhsT[16j+e, 32g+4j+b] = capsT[16b+e, 8g+j]
                # 32 DMAs (one per j,b); flat dims (e, g) + implicit elem dim
                # (DMA APs max 3 dims with a stride-1 last dim).
                lhsT = big.tile([128, 512], bf16)
                nc.sync.dma_start(lhsT, d_zeros)
                _eng = [nc.sync, nc.scalar]
                for j in range(8):
                    for b in range(4):
                        _eng[(2 * j + b) % 2].dma_start(
                            out=lhsT[16 * j:16 * (j + 1), (4 * j + b)::32],
                            in_=capsT[16 * b:16 * (b + 1), j::8])

                lhsT2 = work.tile([128, 512], f32r, tag="lhsT2")
                nc.vector.tensor_mul(lhsT2, lhsT, lhsT)
                p_nc = ps_tiny.tile([1, 512], f32, tag="pnc")
                nc.tensor.matmul(p_nc, ones128, lhsT2, start=True, stop=True)
                fcols = work.tile([1, 512], f32, tag="fcols", bufs=1)
                nc.vector.tensor_copy(fcols, p_nc)
                rtc = work.tile([1, 512], f32, tag="rtc", bufs=1)
                nc.scalar.activation(out=rtc, in_=fcols, func=AF.Sqrt,
                                     bias=0.0, scale=1.0)
                denc = work.tile([1, 512], f32, tag="denc", bufs=1)
                nc.vector.tensor_scalar_add(denc, fcols, 1.0)
                recc = work.tile([1, 512], f32, tag="recc", bufs=1)
                nc.vector.reciprocal(recc, denc)
                fctc = work.tile([1, 512], f32, tag="fctc", bufs=1)
                nc.vector.tensor_mul(fctc, rtc, recc)
                # fct_pt[p, t] = fctc[0, 128*t + p] (p = 32q+4j+b matches
                # the (g,j,b) column order of lhsT for g = 4t+q)
                fct_pt = work.tile([128, 4], f32, tag="fct_pt", bufs=1)
                for t in range(4):
                    nc.sync.dma_start(fct_pt[:, t:t + 1],
                                      fctc[0:1, 128 * t:128 * (t + 1)])
                if debug and rep == 0:
                    nc.gpsimd.dma_start(out=d_dbg["lhsT"], in_=lhsT)
                # u matmuls: 16 groups of 8 capsules; 4 groups col-tiled
                # per psum tile. u_all[p = 32q + 4j + b, (t, o, v)] in bf16,
                # i_local = 8*(4t+q) + j
                u_all = big.tile([128, 4096], bf16)
                for t in range(4):
                    pu = ps_b.tile([128, 1024], f32, tag="pu")
                    for q in range(4):
                        g = 4 * t + q
                        for h in range(2):
                            nc.tensor.matmul(
                                pu[32 * q:32 * q + 32, bass.ts(h, 512)],
                                lhsT[:, bass.ts(g, 32)],
                                wc_sb[g][:, bass.ts(h, 512)],
                                start=True, stop=True,
                                tile_position=(0, 32 * q))
                    if t % 2 == 0:
                        nc.vector.tensor_scalar_mul(
                            u_all[:, bass.ts(t, 1024)], pu,
                            fct_pt[:, t:t + 1])
                    else:
                        nc.scalar.activation(
                            out=u_all[:, bass.ts(t, 1024)], in_=pu,
                            func=AF.Copy, bias=0.0,
                            scale=fct_pt[:, t:t + 1])
                    if debug and rep == 0:
                        nc.gpsimd.dma_start(out=d_dbg[f"u{t}"],
                                            in_=u_all[:, bass.ts(t, 1024)])

                # ---- routing ----
                b_log = big.tile([128, 128], f32)  # [(q,j,b), (t,o)]
                uv = u_all.rearrange("p (t o v) -> p t o v", t=4, v=32)

                def s_partial(tiles, sel, pst):
                    # pst[4, 1024] = sum_t sel.T @ tiles[:, t] (partition sum
                    # selecting b = p%4); 'sel' also carries the 1/32 of c0.
                    for t in range(4):
                        for h in range(2):
                            nc.tensor.matmul(
                                pst[:, bass.ts(h, 512)], sel,
                                tiles[:, 1024 * t + 512 * h:
                                      1024 * t + 512 * (h + 1)],
                                start=(t == 0), stop=(t == 3))

                def allreduce_s(pst, it):
                    s_loc = small.tile([4, 1024], f32, tag="s_loc")
                    nc.vector.tensor_copy(s_loc, pst)
                    nc.sync.dma_start(d_s_in[rep][it].ap(), s_loc)
                    if nocoll:
                        nc.sync.dma_start(d_s_out[rep][it].ap(),
                                          d_s_in[rep][it].ap())
                    else:
                        nc.gpsimd.collective_compute(
                            "AllReduce", AL.add,
                            ins=[d_s_in[rep][it].ap().opt()],
                            outs=[d_s_out[rep][it].ap().opt()],
                            replica_groups=rg)
                    s_glob = small.tile([4, 1024], f32, tag=f"s_glob{it}")
                    nc.sync.dma_start(s_glob, d_s_out[rep][it].ap())
                    return s_glob

                def squash_apply(s_glob, tag="sq_a"):
                    # a = s * |s|/(1+|s|^2) per (b, o); new f32r tile
                    s2 = small.tile([4, 1024], f32, tag="sq_s2")
                    nc.scalar.square(s2, s_glob)
                    sn2 = small.tile([4, 32], f32, tag="sq_n2")
                    nc.vector.reduce_sum(
                        sn2, s2.rearrange("p (o v) -> p o v", v=32), axis=AX.X)
                    srt = small.tile([4, 32], f32, tag="sq_rt")
                    nc.scalar.activation(out=srt, in_=sn2, func=AF.Sqrt,
                                         bias=0.0, scale=1.0)
                    sden = small.tile([4, 32], f32, tag="sq_den")
                    nc.vector.tensor_scalar_add(sden, sn2, 1.0)
                    srec = small.tile([4, 32], f32, tag="sq_rec")
                    nc.vector.reciprocal(srec, sden)
                    sf = small.tile([4, 32], f32, tag="sq_f")
                    nc.vector.tensor_mul(sf, srt, srec)
                    a_r = small.tile([4, 1024], f32r, tag=tag)
                    nc.vector.tensor_mul(
                        a_r.rearrange("p (o v) -> p o v", v=32),
                        s_glob.rearrange("p (o v) -> p o v", v=32),
                        sf.unsqueeze(2).broadcast_to((4, 32, 32)))
                    return a_r

                def agree_update(a, first):
                    # b_log[p, (t,o)] (+)= sum_v a_bc[p, (o,v)] * u[p,(t,o,v)]
                    p_abc = ps_b.tile([128, 1024], f32, tag="pu")
                    for h in range(2):
                        nc.tensor.matmul(p_abc[:, bass.ts(h, 512)], sel4to128,
                                         a[:, bass.ts(h, 512)],
                                         start=True, stop=True)
                    abc_sb = work.tile([128, 1024], bf16, tag="abc")
                    nc.scalar.copy(abc_sb, p_abc)
                    abc_bc = abc_sb.rearrange("p (o v) -> p o v", v=32)
                    for t in range(4):
                        tmp = work.tile([128, 1024], bf16, tag="tmp")
                        tv = tmp.rearrange("p (o v) -> p o v", v=32)
                        nc.vector.tensor_mul(
                            tv, uv[:, t, :, :], abc_bc)
                        if first:
                            nc.vector.reduce_sum(
                                b_log[:, bass.ts(t, 32)], tv, axis=AX.X)
                        else:
                            agr = work.tile([128, 32], f32, tag="agr")
                            nc.vector.reduce_sum(agr, tv, axis=AX.X)
                            nc.vector.tensor_add(b_log[:, bass.ts(t, 32)],
                                                 b_log[:, bass.ts(t, 32)],
                                                 agr)

                def softmax_c():
                    cexp = work.tile([128, 128], f32, tag="cexp")
                    nc.scalar.activation(out=cexp, in_=b_log, func=AF.Exp,
                                         bias=0.0, scale=1.0)
                    sums = small.tile([128, 4], f32, tag="csum")
                    nc.vector.reduce_sum(
                        sums, cexp.rearrange("p (t o) -> p t o", o=32),
                        axis=AX.X)
                    crec = small.tile([128, 4], f32, tag="crec")
                    nc.vector.reciprocal(crec, sums)
                    c_sb = work.tile([128, 128], bf16, tag="c_sb")
                    nc.vector.tensor_mul(
                        c_sb.rearrange("p (t o) -> p t o", o=32),
                        cexp.rearrange("p (t o) -> p t o", o=32),
                        crec.unsqueeze(2).broadcast_to((128, 4, 32)))
                    return c_sb

                def weighted_tiles(c_sb):
                    wt = work.tile([128, 4096], bf16, tag="wt")
                    wv = wt.rearrange("p (t o v) -> p t o v", t=4, v=32)
                    for t in range(4):
                        nc.vector.tensor_mul(
                            wv[:, t, :, :], uv[:, t, :, :],
                            c_sb[:, bass.ts(t, 32)].unsqueeze(2)
                                .broadcast_to((128, 32, 32)))
                    return wt

                # iteration 0: c uniform = 1/32 -> s0 = sum_i u/32 directly on PE
                ps0 = ps_s.tile([4, 1024], f32, tag="ps")
                s_partial(u_all, sel132, ps0)
                sg0 = allreduce_s(ps0, 0)
                if debug and rep == 0:
                    nc.sync.dma_start(d_dbg["sg0"], sg0)
                a0 = squash_apply(sg0)
                agree_update(a0, first=True)
                if debug and rep == 0:
                    nc.sync.dma_start(d_dbg["a0"], a0.bitcast(f32))
                    nc.sync.dma_start(d_dbg["blog0"], b_log)

                # iteration 1
                c1it = softmax_c()
                if debug and rep == 0:
                    nc.sync.dma_start(d_dbg["c1it"], c1it)
                wt1 = weighted_tiles(c1it)
                ps1 = ps_s.tile([4, 1024], f32, tag="ps")
                s_partial(wt1, sel1, ps1)
                sg1 = allreduce_s(ps1, 1)
                if debug and rep == 0:
                    nc.sync.dma_start(d_dbg["sg1"], sg1)
                a1 = squash_apply(sg1)
                agree_update(a1, first=False)

                # iteration 2 (final): s only, squash -> out
                wt2 = weighted_tiles(softmax_c())
                ps2 = ps_s.tile([4, 1024], f32, tag="ps")
                s_partial(wt2, sel1, ps2)
                out_sb = squash_apply(allreduce_s(ps2, 2))
                nc.sync.dma_start(
                    d_out, out_sb.bitcast(f32).rearrange("p (o v) -> p o v", v=32))

        for _rep in range(reps):
            _body(_rep)


    nc.compile()
    return nc


def _prepare_inputs(x, w1, g1, b1, m1, v1, w2, g2, b2, m2, v2,
                    wb, gb, bb, mb, vb, Wc):
    """Host-side: fold BN into weights, transpose/shard for the device."""
    fl = np.float32
    x = np.asarray(x, fl); w1 = np.asarray(w1, fl); w2 = np.asarray(w2, fl)
    wb = np.asarray(wb, fl); Wc = np.asarray(Wc, fl)
    g1, b1, m1, v1 = (np.asarray(a, fl) for a in (g1, b1, m1, v1))
    g2, b2, m2, v2 = (np.asarray(a, fl) for a in (g2, b2, m2, v2))
    gb, bb, mb, vb = (np.asarray(a, fl) for a in (gb, bb, mb, vb))

    s1 = g1 / np.sqrt(v1 + EPS)
    c1 = b1 - m1 * s1
    w1f = (w1 * s1[:, None]).T.copy()            # [3, 64]
    c1f = np.ascontiguousarray(c1[:, None])

    s2 = g2 / np.sqrt(v2 + EPS)
    c2 = b2 - m2 * s2
    w2f = (w2 * s2[:, None]).T.copy()            # [64, 128]
    c2f = np.ascontiguousarray(c2[:, None])

    sb = gb / np.sqrt(vb + EPS)                  # [16, 1024]
    wbp = wb * sb[:, :, None]                    # [16, 1024, 128]
    cbv = bb - mb * sb                           # [16, 1024]

    xT = np.ascontiguousarray(x.reshape(BN_, 3).T)  # [3, 8192]

    p = np.arange(128)
    sel1 = ((p[:, None] % 4) == np.arange(4)[None, :]).astype(fl)
    sel132 = sel1 / 32.0
    sel4to128 = np.ascontiguousarray(sel1.T)
    e64 = np.arange(64)
    selsq = ((e64[:, None] // 16) == np.arange(4)[None, :]).astype(fl)
    sel4to64 = np.ascontiguousarray(selsq.T)

    shared = {
        "xT": xT, "w1f": w1f, "c1f": c1f, "w2f": w2f, "c2f": c2f,
        "sel132": sel132.astype(_BF), "sel1": sel1.astype(_BF),
        "sel4to128": sel4to128,
        "selsq": selsq, "sel4to64": sel4to64,
        "zeros512": np.zeros((128, 512), _BF),
        "ident128": np.eye(128, dtype=fl),
        "ones128": np.ones((128, 1), fl),
    }

    in_maps = []
    for c in range(N_CORES):
        m = dict(shared)
        ks = slice(2 * c, 2 * c + 2)
        # wbT[p=ch, (k, oc, o)] = wbp[2c+k, 128*oc+o, ch]
        m["wbT"] = np.ascontiguousarray(
            wbp[ks].reshape(2, 8, 128, 128).transpose(3, 0, 1, 2)
            .reshape(128, 2048))
        # cb[p, (oc, k)] = cbv[2c+k, 128*oc+p]
        m["cb"] = np.ascontiguousarray(
            cbv[ks].reshape(2, 8, 128).transpose(2, 1, 0).reshape(128, 16))
        # wc[g, 16j+e, 32o+v] = Wc[o, 128c+8g+j, e, v]
        wcs = Wc[:, 128 * c:128 * (c + 1)]       # [32, 128, 16, 32]
        m["wc"] = np.ascontiguousarray(
            wcs.reshape(32, 16, 8, 16, 32)       # [o, g, j, e, v]
            .transpose(1, 2, 3, 0, 4)            # [g, j, e, o, v]
            .reshape(16, 128, 1024)).astype(_BF)
        in_maps.append(m)
    return in_maps


def kernel(**inputs):
    if "nc" not in _CACHE:
        _CACHE["nc"] = _build_bass()
    nc = _CACHE["nc"]
    in_maps = _prepare_inputs(**inputs)
    res = bass_utils.run_bass_kernel_spmd(
        nc, in_maps, core_ids=list(range(N_CORES)))
    return np.asarray(res.results[0]["out"], dtype=np.float32)



# revision 28
# speedup vs baseline: 1.0082x; 1.0082x over previous
"""Trainium2 Bass kernel for nn_Encoder_55688545960036.

Network: pointnet-style shared MLP (3->64->128, eval-mode BN folded into the
weights on the host, relu), 16 branch matmuls 128->1024 with folded BN and a
maxpool over the 2048 points of each batch element, squash over the branch
axis, capsule transform u[b,o,i,v] = sum_e caps[b,i,e] * Wc[o,i,e,v], 3 rounds
of dynamic routing, squash -> [4,32,32] output.

Distribution over 8 NeuronCores:
  phase A: branch axis k=16 -> 2 branches per core (shared MLP replicated).
  phase B: in-capsule axis i=1024 -> 128 per core (Wc 67MB -> 8.4MB/core).
  collectives: one AllToAll that converts per-core (2 branches, all 1024 i)
  feat into per-core (all 16 branches, 128-i shard), then 3 AllReduces of the
  routing partial sums s[4,32,32] (one per routing iteration). The AllReduce
  outputs use Local (not Shared) DRAM: Shared-output collectives measured
  ~158us each on this fabric vs ~17us for Local.

All heavy matmuls run in float32r (full PE rate, ~1.5e-4 rel err on HW).
The maxpool is fused into PSUM evacuation with tensor_scalar accum max.
The capsule einsum is done as 16 block-diagonal matmuls (8 capsules of the
i-shard per group, caps entries on the block diagonal) with PE column tiling.
Routing keeps (i,b) on partitions and (o,v) in the free dims so the softmax
over o is a free-axis op; sums over i (partitions) are tiny selector matmuls.
"""

import numpy as np
from contextlib import ExitStack

import concourse.bass as bass
import concourse.tile as tile
from concourse import bacc, mybir
from concourse import bass_utils

EPS = 1e-5
N_CORES = 8
B = 4
BN_ = 4 * 2048  # 8192 points
f32 = mybir.dt.float32
f32r = mybir.dt.float32r
AL = mybir.AluOpType
AF = mybir.ActivationFunctionType
AX = mybir.AxisListType
bf16 = mybir.dt.bfloat16
_BF = mybir.dt.np(bf16)

_CACHE = {}

# maxpool block schedule: 64 blocks of (k, oc, b), each with 2048 points in
# two [128,1024] PSUM tiles. Path V = DVE tensor_tensor max of both psum
# tiles (one instruction reads both) -> bf16 SBUF + remax; path A = 2 ACT
# copies -> bf16 SBUF + remax. Remax engine: 'v' = DVE (bf16 4x), 'p' =
# GpSimd/Pool. Tuned so DVE/ACT/Pool busy are balanced.
_W5 = ["D", "A", "A", "A", "D", "A", "A", "D",
       "A", "A", "A", "D", "A", "A", "D", "A"]
_W6 = ["D", "D", "A", "A", "D", "A", "A", "D",
       "A", "A", "A", "D", "A", "A", "D", "A"]
# 21 D per 64 tiles (42 of 128): balances DVE ~120 / ACT ~119 modeled busy
MAXPOOL_PATHS = _W5 + _W6 + _W5 + _W5


def _build_bass(reps=1, debug=False, stage=4, nocoll=False):
    # stage: 1=MLP only, 2=+branch/maxpool, 3=+A2A/caps/u, 4=full (routing)
    # nocoll: replace collectives with local DRAM copies (for TimelineSim)
    # reps > 1 replicates the compute body end-to-end inside one NEFF; used
    # only for wall-clock-difference timing in the dev harness.
    nc = bacc.Bacc("TRN2", target_bir_lowering=False, debug=False,
                   num_devices=N_CORES)

    # ---- DRAM I/O ----
    d_xT = nc.dram_tensor("xT", [3, BN_], f32r, kind="ExternalInput").ap()
    d_w1f = nc.dram_tensor("w1f", [3, 64], f32r, kind="ExternalInput").ap()
    d_c1f = nc.dram_tensor("c1f", [64, 1], f32, kind="ExternalInput").ap()
    d_w2f = nc.dram_tensor("w2f", [64, 128], f32r, kind="ExternalInput").ap()
    d_c2f = nc.dram_tensor("c2f", [128, 1], f32, kind="ExternalInput").ap()
    d_wbT = nc.dram_tensor("wbT", [128, 2048], f32r, kind="ExternalInput").ap()
    d_cb = nc.dram_tensor("cb", [128, 16], f32, kind="ExternalInput").ap()
    d_wc = nc.dram_tensor("wc", [16, 128, 1024], bf16, kind="ExternalInput").ap()
    d_sel132 = nc.dram_tensor("sel132", [128, 4], bf16, kind="ExternalInput").ap()
    d_sel1 = nc.dram_tensor("sel1", [128, 4], bf16, kind="ExternalInput").ap()
    d_sel4to128 = nc.dram_tensor("sel4to128", [4, 128], f32r,
                                 kind="ExternalInput").ap()
    d_selsq = nc.dram_tensor("selsq", [64, 4], f32r, kind="ExternalInput").ap()
    d_sel4to64 = nc.dram_tensor("sel4to64", [4, 64], f32r,
                                kind="ExternalInput").ap()
    d_zeros = nc.dram_tensor("zeros512", [128, 512], bf16,
                             kind="ExternalInput").ap()
    d_ident = nc.dram_tensor("ident128", [128, 128], f32,
                             kind="ExternalInput").ap()
    d_ones = nc.dram_tensor("ones128", [128, 1], f32r,
                            kind="ExternalInput").ap()
    d_out = nc.dram_tensor("out", [B, 32, 32], f32, kind="ExternalOutput").ap()
    d_dbg = {}
    if debug:
        for nm, shp in [("h2T", [128, BN_]), ("feat", [128, 64]),
                        ("FT", [64, 128]), ("capsT", [64, 128]),
                        ("lhsT", [128, 512]), ("u0", [128, 1024]),
                        ("u1", [128, 1024]), ("u2", [128, 1024]),
                        ("u3", [128, 1024]), ("blog0", [128, 128]),
                        ("sg0", [4, 1024]), ("a0", [4, 1024]),
                        ("c1it", [128, 128]), ("sg1", [4, 1024])]:
            d_dbg[nm] = nc.dram_tensor("dbg_" + nm, shp, f32,
                                       kind="ExternalOutput").ap()

    # collective bounce buffers (internal DRAM); A2A operates on first-dim
    # blocks: in[j] goes to rank j, out[r] came from rank r.
    d_a2a_in = [nc.dram_tensor(f"a2a_in_r{r}", [8, 2, B, 128], f32)
                for r in range(reps)]
    d_a2a_out = [nc.dram_tensor(f"a2a_out_r{r}", [8, 2, B, 128], f32)
                 for r in range(reps)]
    d_s_in = [[nc.dram_tensor(f"s_in{t}_r{r}", [B, 1024], f32)
               for t in range(3)] for r in range(reps)]
    d_s_out = [[nc.dram_tensor(f"s_out{t}_r{r}", [B, 1024], f32)
               for t in range(3)] for r in range(reps)]

    rg = [list(range(N_CORES))]

    with tile.TileContext(nc) as tc, ExitStack() as ctx:
        const = ctx.enter_context(tc.tile_pool(name="const", bufs=1))
        big = ctx.enter_context(tc.tile_pool(name="big", bufs=1))
        work = ctx.enter_context(tc.tile_pool(name="work", bufs=2))
        small = ctx.enter_context(tc.tile_pool(name="small", bufs=1))

        # ---- load constants / weights ----
        def load_const(name, dram, shape, dt, eng=None):
            t = const.tile(shape, dt, name=name)
            (eng or nc.sync).dma_start(out=t, in_=dram)
            return t

        xT = load_const("xT_sb", d_xT, [3, BN_], f32r)
        w1f = load_const("w1f_sb", d_w1f, [3, 64], f32r)
        c1f = load_const("c1f_sb", d_c1f, [64, 1], f32)
        w2f = load_const("w2f_sb", d_w2f, [64, 128], f32r)
        c2f = load_const("c2f_sb", d_c2f, [128, 1], f32)
        wbT = load_const("wbT_sb", d_wbT, [128, 2048], f32r)
        cb = load_const("cb_sb", d_cb, [128, 16], f32)
        sel132 = load_const("sel132_sb", d_sel132, [128, 4], bf16)
        sel1 = load_const("sel1_sb", d_sel1, [128, 4], bf16)
        sel4to128 = load_const("sel4to128_sb", d_sel4to128, [4, 128], f32r)
        selsq = load_const("selsq_sb", d_selsq, [64, 4], f32r)
        sel4to64 = load_const("sel4to64_sb", d_sel4to64, [4, 64], f32r)

        ident = load_const("ident_sb", d_ident, [128, 128], f32)
        ones128 = load_const("ones128_sb", d_ones, [128, 1], f32r)
        wc_sb = [load_const(f"wc_sb{g}", d_wc[g], [128, 1024], bf16,
                            eng=nc.gpsimd)
                 for g in range(16)]

        def _body(rep):
            h2T = big.tile([128, BN_], f32r)  # [channel, point]

            # ---- phase A: shared MLP ----
            # 1024-wide psum tiles (2 matmuls each, matmul outputs stay
            # within a 512-col bank) halve the evacuation-op count.
            with tc.tile_pool(name="ps_mlp", bufs=2, space="PSUM") as ps_mlp:
                for j in range(8):
                    sl = bass.ts(j, 1024)
                    p1 = ps_mlp.tile([64, 1024], f32, tag="p1")
                    for q2 in range(2):
                        nc.tensor.matmul(p1[:, bass.ts(q2, 512)], w1f,
                                         xT[:, bass.ts(2 * j + q2, 512)],
                                         start=True, stop=True)
                    h1c = work.tile([64, 1024], f32r, tag="h1c", bufs=2)
                    nc.scalar.activation(out=h1c, in_=p1, func=AF.Relu,
                                         bias=c1f, scale=1.0)
                    p2 = ps_mlp.tile([128, 1024], f32, tag="p2")
                    for q2 in range(2):
                        nc.tensor.matmul(p2[:, bass.ts(q2, 512)], w2f,
                                         h1c[:, bass.ts(q2, 512)],
                                         start=True, stop=True)
                    if j % 2 == 0:
                        nc.scalar.activation(out=h2T[:, sl], in_=p2,
                                             func=AF.Relu, bias=c2f, scale=1.0)
                    else:
                        nc.vector.tensor_scalar(out=h2T[:, sl], in0=p2,
                                                scalar1=c2f, scalar2=0.0,
                                                op0=AL.add, op1=AL.max)

            if stage <= 1:
                nc.sync.dma_start(
                    d_out, h2T[0:B, 0:1024].bitcast(f32)
                    .rearrange("p (o v) -> p o v", v=32))
                return
            # ---- phase A: branch matmuls + fused maxpool ----
            feat_sb = big.tile([128, 64], f32)  # [o_in_chunk, (k, oc, b)]
            feat_pt = big.tile([128, 128], f32)  # per-half partials
            with tc.tile_pool(name="ps_y", bufs=4, space="PSUM") as ps_y:
                idx = 0
                for k in range(2):
                    for oc in range(8):
                        lw = wbT[:, bass.ts(k * 8 + oc, 128)]
                        for b in range(B):
                            for h in range(2):
                                py = ps_y.tile([128, 1024], f32, tag="py")
                                for q2 in range(2):
                                    q = 2 * h + q2
                                    nc.tensor.matmul(
                                        py[:, bass.ts(q2, 512)], lw,
                                        h2T[:, bass.ts(4 * b + q, 512)],
                                        start=True, stop=True)
                                s2 = ((oc * 2 + k) * 4 + b) * 2 + h
                                acc = feat_pt[:, s2:s2 + 1]
                                path = MAXPOOL_PATHS[idx % len(MAXPOOL_PATHS)]
                                if path == "D":
                                    nc.vector.tensor_scalar(
                                        out=py, in0=py, scalar1=-3.0e38,
                                        scalar2=None, op0=AL.max, op1=AL.max,
                                        accum_out=acc)
                                else:
                                    pair = work.tile([128, 1024], bf16,
                                                     tag="pair", bufs=8)
                                    nc.scalar.copy(pair, py)
                                    eng = (nc.vector if path == "A"
                                           else nc.gpsimd)
                                    eng.tensor_scalar(
                                        out=pair, in0=pair, scalar1=-3.0e38,
                                        scalar2=None, op0=AL.max, op1=AL.max,
                                        accum_out=acc)
                                idx += 1
            nc.vector.tensor_max(feat_sb, feat_pt[:, 0::2], feat_pt[:, 1::2])

            # feat += cb (cb[p, (oc, k)] broadcast over b)
            feat_v = feat_sb.rearrange("p (oc k b) -> p oc k b", oc=8, k=2)
            cb_bc = cb.rearrange("p (oc k) -> p oc k", oc=8).unsqueeze(3) \
                      .broadcast_to((128, 8, 2, 4))
            nc.vector.tensor_add(feat_v, feat_v, cb_bc)

            if debug and rep == 0:
                nc.sync.dma_start(d_dbg["feat"], feat_sb)
            if stage <= 2:
                nc.sync.dma_start(
                    d_out[:, 0:4, 0:8],
                    feat_sb[0:B, 0:32].rearrange("p (o v) -> p o v", v=8))
                return
            # transpose feat on the PE so the a2a_in DMA is one contiguous
            # 32KB copy (featT flat layout == a2a_in flat layout).
            with tc.tile_pool(name="ps_ft", bufs=1, space="PSUM") as ps_ft:
                p_ftr = ps_ft.tile([64, 128], f32, tag="pft")
                nc.tensor.transpose(p_ftr, feat_sb, ident)
                featT = work.tile([64, 128], f32, tag="featT")
                nc.vector.tensor_copy(featT, p_ftr)
            nc.sync.dma_start(d_a2a_in[rep].ap(), featT)

            # ---- AllToAll: out viewed [16(e), B, 128(i_local)] ----
            if nocoll:
                nc.sync.dma_start(d_a2a_out[rep].ap(), d_a2a_in[rep].ap())
            else:
                nc.gpsimd.collective_compute(
                    "AllToAll", AL.bypass, ins=[d_a2a_in[rep].ap().opt()],
                    outs=[d_a2a_out[rep].ap().opt()], replica_groups=rg)

            # ---- phase B ----
            with tc.tile_pool(name="ps_b", bufs=2, space="PSUM") as ps_b, \
                 tc.tile_pool(name="ps_tiny", bufs=2, space="PSUM") as ps_tiny, \
                 tc.tile_pool(name="ps_s", bufs=1, space="PSUM") as ps_s:

                # caps: squash over branch axis e. FT[(b,e) b-major, i_local]:
                # FT[16b+e, i] = a2a_out[e, b, i]; one DMA per b (custom
                # multi-dim APs on internal DRAM tensors get re-normalized,
                # so keep each transfer 2-dim and unambiguous).
                FT = big.tile([64, 128], f32)
                a2a_out_v = d_a2a_out[rep].ap().rearrange(
                    "r k b l -> (r k) b l")  # [16 e, 4 b, 128 i]
                for b in range(4):
                    nc.sync.dma_start(FT[16 * b:16 * (b + 1), :],
                                      a2a_out_v[:, b, :])

                # u is computed from UNNORMALIZED caps; the squash scale
                # fct[b,i] is derived from column-sums of lhsT^2 (each
                # block-diag column holds exactly one capsule vector) and
                # folded into the u evacuation, off the critical path.
                capsT = work.tile([64, 128], bf16, tag="capsT")
                nc.vector.tensor_copy(capsT, FT)
                if debug and rep == 0:
                    nc.sync.dma_start(d_dbg["FT"], FT)
                    nc.gpsimd.dma_start(out=d_dbg["capsT"], in_=capsT)

                # block-diagonal lhsT: lhsT[16j+e, 32g+4j+b] = capsT[16b+e, 8g+j]
                # 32 DMAs (one per j,b); flat dims (e, g) + implicit elem dim
                # (DMA APs max 3 dims with a stride-1 last dim).
                lhsT = big.tile([128, 512], bf16)
                nc.sync.dma_start(lhsT, d_zeros)
                _eng = [nc.sync, nc.scalar]
                for j in range(8):
                    for b in range(4):
                        _eng[(2 * j + b) % 2].dma_start(
                            out=lhsT[16 * j:16 * (j + 1), (4 * j + b)::32],
                            in_=capsT[16 * b:16 * (b + 1), j::8])

                lhsT2 = work.tile([128, 512], f32r, tag="lhsT2")
                nc.vector.tensor_mul(lhsT2, lhsT, lhsT)
                p_nc = ps_tiny.tile([1, 512], f32, tag="pnc")
                nc.tensor.matmul(p_nc, ones128, lhsT2, start=True, stop=True)
                fcols = work.tile([1, 512], f32, tag="fcols", bufs=1)
                nc.vector.tensor_copy(fcols, p_nc)
                rtc = work.tile([1, 512], f32, tag="rtc", bufs=1)
                nc.scalar.activation(out=rtc, in_=fcols, func=AF.Sqrt,
                                     bias=0.0, scale=1.0)
                denc = work.tile([1, 512], f32, tag="denc", bufs=1)
                nc.vector.tensor_scalar_add(denc, fcols, 1.0)
                recc = work.tile([1, 512], f32, tag="recc", bufs=1)
                nc.vector.reciprocal(recc, denc)
                fctc = work.tile([1, 512], f32, tag="fctc", bufs=1)
                nc.vector.tensor_mul(fctc, rtc, recc)
                # fct_pt[p, t] = fctc[0, 128*t + p] (p = 32q+4j+b matches
                # the (g,j,b) column order of lhsT for g = 4t+q)
                fct_pt = work.tile([128, 4], f32, tag="fct_pt", bufs=1)
                for t in range(4):
                    nc.sync.dma_start(fct_pt[:, t:t + 1],
                                      fctc[0:1, 128 * t:128 * (t + 1)])
                if debug and rep == 0:
                    nc.gpsimd.dma_start(out=d_dbg["lhsT"], in_=lhsT)
                # u matmuls: 16 groups of 8 capsules; 4 groups col-tiled
                # per psum tile. u_all[p = 32q + 4j + b, (t, o, v)] in bf16,
                # i_local = 8*(4t+q) + j
                u_all = big.tile([128, 4096], bf16)
                for t in range(4):
                    pu = ps_b.tile([128, 1024], f32, tag="pu")
                    for q in range(4):
                        g = 4 * t + q
                        for h in range(2):
                            nc.tensor.matmul(
                                pu[32 * q:32 * q + 32, bass.ts(h, 512)],
                                lhsT[:, bass.ts(g, 32)],
                                wc_sb[g][:, bass.ts(h, 512)],
                                start=True, stop=True,
                                tile_position=(0, 32 * q))
                    if t % 2 == 0:
                        nc.vector.tensor_scalar_mul(
                            u_all[:, bass.ts(t, 1024)], pu,
                            fct_pt[:, t:t + 1])
                    else:
                        nc.scalar.activation(
                            out=u_all[:, bass.ts(t, 1024)], in_=pu,
                            func=AF.Copy, bias=0.0,
                            scale=fct_pt[:, t:t + 1])
                    if debug and rep == 0:
                        nc.gpsimd.dma_start(out=d_dbg[f"u{t}"],
                                            in_=u_all[:, bass.ts(t, 1024)])

                # ---- routing ----
                b_log = big.tile([128, 128], f32)  # [(q,j,b), (t,o)]
                uv = u_all.rearrange("p (t o v) -> p t o v", t=4, v=32)

                def s_partial(tiles, sel, pst):
                    # pst[4, 1024] = sum_t sel.T @ tiles[:, t] (partition sum
                    # selecting b = p%4); 'sel' also carries the 1/32 of c0.
                    for t in range(4):
                        for h in range(2):
                            nc.tensor.matmul(
                                pst[:, bass.ts(h, 512)], sel,
                                tiles[:, 1024 * t + 512 * h:
                                      1024 * t + 512 * (h + 1)],
                                start=(t == 0), stop=(t == 3))

                def allreduce_s(pst, it):
                    s_loc = small.tile([4, 1024], f32, tag="s_loc")
                    nc.scalar.copy(s_loc, pst)
                    nc.sync.dma_start(d_s_in[rep][it].ap(), s_loc)
                    if nocoll:
                        nc.sync.dma_start(d_s_out[rep][it].ap(),
                                          d_s_in[rep][it].ap())
                    else:
                        nc.gpsimd.collective_compute(
                            "AllReduce", AL.add,
                            ins=[d_s_in[rep][it].ap().opt()],
                            outs=[d_s_out[rep][it].ap().opt()],
                            replica_groups=rg)
                    s_glob = small.tile([4, 1024], f32, tag=f"s_glob{it}")
                    nc.sync.dma_start(s_glob, d_s_out[rep][it].ap())
                    return s_glob

                def squash_apply(s_glob, tag="sq_a"):
                    # a = s * |s|/(1+|s|^2) per (b, o); new f32r tile
                    s2 = small.tile([4, 1024], f32, tag="sq_s2")
                    nc.scalar.square(s2, s_glob)
                    sn2 = small.tile([4, 32], f32, tag="sq_n2")
                    nc.vector.reduce_sum(
                        sn2, s2.rearrange("p (o v) -> p o v", v=32), axis=AX.X)
                    srt = small.tile([4, 32], f32, tag="sq_rt")
                    nc.scalar.activation(out=srt, in_=sn2, func=AF.Sqrt,
                                         bias=0.0, scale=1.0)
                    sden = small.tile([4, 32], f32, tag="sq_den")
                    nc.vector.tensor_scalar_add(sden, sn2, 1.0)
                    srec = small.tile([4, 32], f32, tag="sq_rec")
                    nc.vector.reciprocal(srec, sden)
                    sf = small.tile([4, 32], f32, tag="sq_f")
                    nc.vector.tensor_mul(sf, srt, srec)
                    a_r = small.tile([4, 1024], f32r, tag=tag)
                    nc.vector.tensor_mul(
                        a_r.rearrange("p (o v) -> p o v", v=32),
                        s_glob.rearrange("p (o v) -> p o v", v=32),
                        sf.unsqueeze(2).broadcast_to((4, 32, 32)))
                    return a_r

                def agree_update(a, first):
                    # b_log[p, (t,o)] (+)= sum_v a_bc[p, (o,v)] * u[p,(t,o,v)]
                    p_abc = ps_b.tile([128, 1024], f32, tag="pu")
                    for h in range(2):
                        nc.tensor.matmul(p_abc[:, bass.ts(h, 512)], sel4to128,
                                         a[:, bass.ts(h, 512)],
                                         start=True, stop=True)
                    abc_sb = work.tile([128, 1024], bf16, tag="abc")
                    nc.scalar.copy(abc_sb, p_abc)
                    abc_bc = abc_sb.rearrange("p (o v) -> p o v", v=32)
                    agr_all = (None if first else
                               work.tile([128, 128], f32, tag="agr"))
                    for t in range(4):
                        tmp = work.tile([128, 1024], bf16, tag="tmp")
                        tv = tmp.rearrange("p (o v) -> p o v", v=32)
                        nc.vector.tensor_mul(
                            tv, uv[:, t, :, :], abc_bc)
                        if first:
                            nc.vector.reduce_sum(
                                b_log[:, bass.ts(t, 32)], tv, axis=AX.X)
                        else:
                            nc.vector.reduce_sum(
                                agr_all[:, bass.ts(t, 32)], tv, axis=AX.X)
                    if not first:
                        nc.vector.tensor_add(b_log, b_log, agr_all)

                def softmax_c():
                    cexp = work.tile([128, 128], f32, tag="cexp")
                    nc.scalar.activation(out=cexp, in_=b_log, func=AF.Exp,
                                         bias=0.0, scale=1.0)
                    sums = small.tile([128, 4], f32, tag="csum")
                    nc.vector.reduce_sum(
                        sums, cexp.rearrange("p (t o) -> p t o", o=32),
                        axis=AX.X)
                    crec = small.tile([128, 4], f32, tag="crec")
                    nc.vector.reciprocal(crec, sums)
                    c_sb = work.tile([128, 128], bf16, tag="c_sb")
                    nc.vector.tensor_mul(
                        c_sb.rearrange("p (t o) -> p t o", o=32),
                        cexp.rearrange("p (t o) -> p t o", o=32),
                        crec.unsqueeze(2).broadcast_to((128, 4, 32)))
                    return c_sb

                def weighted_tiles(c_sb):
                    wt = work.tile([128, 4096], bf16, tag="wt")
                    wv = wt.rearrange("p (t o v) -> p t o v", t=4, v=32)
                    for t in range(4):
                        nc.vector.tensor_mul(
                            wv[:, t, :, :], uv[:, t, :, :],
                            c_sb[:, bass.ts(t, 32)].unsqueeze(2)
                                .broadcast_to((128, 32, 32)))
                    return wt

                # iteration 0: c uniform = 1/32 -> s0 = sum_i u/32 directly on PE
                ps0 = ps_s.tile([4, 1024], f32, tag="ps")
                s_partial(u_all, sel132, ps0)
                sg0 = allreduce_s(ps0, 0)
                if debug and rep == 0:
                    nc.sync.dma_start(d_dbg["sg0"], sg0)
                a0 = squash_apply(sg0)
                agree_update(a0, first=True)
                if debug and rep == 0:
                    nc.sync.dma_start(d_dbg["a0"], a0.bitcast(f32))
                    nc.sync.dma_start(d_dbg["blog0"], b_log)

                # iteration 1
                c1it = softmax_c()
                if debug and rep == 0:
                    nc.sync.dma_start(d_dbg["c1it"], c1it)
                wt1 = weighted_tiles(c1it)
                ps1 = ps_s.tile([4, 1024], f32, tag="ps")
                s_partial(wt1, sel1, ps1)
                sg1 = allreduce_s(ps1, 1)
                if debug and rep == 0:
                    nc.sync.dma_start(d_dbg["sg1"], sg1)
                a1 = squash_apply(sg1)
                agree_update(a1, first=False)

                # iteration 2 (final): s only, squash -> out
                wt2 = weighted_tiles(softmax_c())
                ps2 = ps_s.tile([4, 1024], f32, tag="ps")
                s_partial(wt2, sel1, ps2)
                out_sb = squash_apply(allreduce_s(ps2, 2))
                nc.sync.dma_start(
                    d_out, out_sb.bitcast(f32).rearrange("p (o v) -> p o v", v=32))

        for _rep in range(reps):
            _body(_rep)


    nc.compile()
    return nc


def _prepare_inputs(x, w1, g1, b1, m1, v1, w2, g2, b2, m2, v2,
                    wb, gb, bb, mb, vb, Wc):
    """Host-side: fold BN into weights, transpose/shard for the device."""
    fl = np.float32
    x = np.asarray(x, fl); w1 = np.asarray(w1, fl); w2 = np.asarray(w2, fl)
    wb = np.asarray(wb, fl); Wc = np.asarray(Wc, fl)
    g1, b1, m1, v1 = (np.asarray(a, fl) for a in (g1, b1, m1, v1))
    g2, b2, m2, v2 = (np.asarray(a, fl) for a in (g2, b2, m2, v2))
    gb, bb, mb, vb = (np.asarray(a, fl) for a in (gb, bb, mb, vb))

    s1 = g1 / np.sqrt(v1 + EPS)
    c1 = b1 - m1 * s1
    w1f = (w1 * s1[:, None]).T.copy()            # [3, 64]
    c1f = np.ascontiguousarray(c1[:, None])

    s2 = g2 / np.sqrt(v2 + EPS)
    c2 = b2 - m2 * s2
    w2f = (w2 * s2[:, None]).T.copy()            # [64, 128]
    c2f = np.ascontiguousarray(c2[:, None])

    sb = gb / np.sqrt(vb + EPS)                  # [16, 1024]
    wbp = wb * sb[:, :, None]                    # [16, 1024, 128]
    cbv = bb - mb * sb                           # [16, 1024]

    xT = np.ascontiguousarray(x.reshape(BN_, 3).T)  # [3, 8192]

    p = np.arange(128)
    sel1 = ((p[:, None] % 4) == np.arange(4)[None, :]).astype(fl)
    sel132 = sel1 / 32.0
    sel4to128 = np.ascontiguousarray(sel1.T)
    e64 = np.arange(64)
    selsq = ((e64[:, None] // 16) == np.arange(4)[None, :]).astype(fl)
    sel4to64 = np.ascontiguousarray(selsq.T)

    shared = {
        "xT": xT, "w1f": w1f, "c1f": c1f, "w2f": w2f, "c2f": c2f,
        "sel132": sel132.astype(_BF), "sel1": sel1.astype(_BF),
        "sel4to128": sel4to128,
        "selsq": selsq, "sel4to64": sel4to64,
        "zeros512": np.zeros((128, 512), _BF),
        "ident128": np.eye(128, dtype=fl),
        "ones128": np.ones((128, 1), fl),
    }

    in_maps = []
    for c in range(N_CORES):
        m = dict(shared)
        ks = slice(2 * c, 2 * c + 2)
        # wbT[p=ch, (k, oc, o)] = wbp[2c+k, 128*oc+o, ch]
        m["wbT"] = np.ascontiguousarray(
            wbp[ks].reshape(2, 8, 128, 128).transpose(3, 0, 1, 2)
            .reshape(128, 2048))
        # cb[p, (oc, k)] = cbv[2c+k, 128*oc+p]
        m["cb"] = np.ascontiguousarray(
            cbv[ks].reshape(2, 8, 128).transpose(2, 1, 0).reshape(128, 16))
        # wc[g, 16j+e, 32o+v] = Wc[o, 128c+8g+j, e, v]
        wcs = Wc[:, 128 * c:128 * (c + 1)]       # [32, 128, 16, 32]
        m["wc"] = np.ascontiguousarray(
            wcs.reshape(32, 16, 8, 16, 32)       # [o, g, j, e, v]
            .transpose(1, 2, 3, 0, 4)            # [g, j, e, o, v]
            .reshape(16, 128, 1024)).astype(_BF)
        in_maps.append(m)
    return in_maps


def kernel(**inputs):
    if "nc" not in _CACHE:
        _CACHE["nc"] = _build_bass()
    nc = _CACHE["nc"]
    in_maps = _prepare_inputs(**inputs)
    res = bass_utils.run_bass_kernel_spmd(
        nc, in_maps, core_ids=list(range(N_CORES)))
    return np.asarray(res.results[0]["out"], dtype=np.float32)

